# revision 13
# baseline (speedup 1.0000x reference)
"""Single-head causal attention (B=8, T=4096, EMB=1024, HEAD=64) on 8 trn2 cores.

Strategy: data-parallel over batch, one batch element per NeuronCore.

The per-core kernel is scalar-exp-bound (softmax exp runs only on the Scalar
engine at 1 col/cycle: causal T^2/2 elements = 67584 cols ~ 56us @1.2GHz), so
the whole kernel is organized to start exp as early as possible and keep the
Scalar engine 100% fed:

  - x is host-packed as [128, KCH=8, T] and DMA'd per t-tile j (512 cols,
    1MB, ~2.9us each) across multiple issue queues.
  - Per tile j: QK^T[128, 512] (8 k-chunk matmuls into 1 PSUM bank; rows
    0:63 = Q^T, rows 64:127 = K^T), cast to bf16, then TWO shift DMAs build
    qk_sb: rows 0:63 = K^T copy, rows 64:127 = Q^T copy. This gives a
    (K^T, Q^T) operand pair at BOTH partition bases 0 and 64, so score
    matmuls alternate between PE row-tiles T0 (base 0) and T8 (base 64)
    and run concurrently (64x128 row tiling, ~2x effective rate).
  - V[4 subtiles, 64] (32 matmuls, stationary = xt chunk) + copy-with-ones.
  - Scores for s-chunks a<=4j+3 vs t-tile j stream into [128,<=1536] PSUM
    units (3 banks x2 buffered); ScalarE exp's each unit (scale=1/8 folded),
    bf16 out into a per-tile P^T buffer (exact causal widths). Diagonal
    128x128 blocks masked by 0/1 multiply after exp.
  - PV: per t-subtile i, chain of i+1 matmuls (stationary P^T slice
    [128,128], moving V-with-ones [128,65]) accumulates [O|rowsum]; chains
    are emitted in small segments interleaved between score units so score
    matmuls never queue behind a long chain. out = O * reciprocal(rowsum).
  - A burst of dummy matmuls at program start ramps the PE HAM clock gate
    to 2.4GHz before the first real matmul.

PSUM budget (8 banks): scores 2x[128,1536] (6) + KQ [128,512] (1) + V/PV
shared (1).
"""

import numpy as np
import ml_dtypes

B, T, EMB, HEAD = 8, 4096, 1024, 64
KCH = EMB // 128          # 8 contraction chunks
NTT = T // 512            # 8 t-tiles of 512
NTS = T // 128            # 32 t-subtiles / s-chunks of 128
UNIT = 1536               # max score-unit width (3 PSUM banks)
WARM_MMS = 12             # PE clock-gate warm-up matmuls
SEG = 6                   # PV chain segment length (matmuls)
BF16 = ml_dtypes.bfloat16

_CACHE = {}


def _tile_slots(j, partials_first=False):
    """[(a, t0, w, pt_off, start_flag), ...] score slots for t-tile j.

    Exact causal widths, packed so every slot either begins at a PSUM bank
    boundary (start=True allowed: clearing the bank wipes nothing live) or
    is the 128-wide slot sharing the bank opened by the 384 one
    (start=False: its elements' has_written bits were cleared by that
    start). Widths 384,128 share a bank; 256 gets its own (tail unused);
    fulls are 512 each. For the last tile the partials go FIRST so the
    final PV chains are gated by the last full unit, not the partials
    (safe only there: partials need this tile's own K-copy)."""
    parts = [(4 * j + 1, True), (4 * j + 3, False), (4 * j + 2, True)]
    fulls = [(a, True) for a in range(4 * j + 1)]
    order = parts + fulls if partials_first else fulls + parts
    slots = []
    off = 0
    for a, flag in order:
        t0 = max(512 * j, 128 * a)
        w = 512 * (j + 1) - t0
        slots.append((a, t0, w, off, flag))
        off += w
    return slots


def _units(slots):
    """Greedy-pack slots into units of <= UNIT psum columns, whole banks.

    A start=True slot opens a PSUM bank, so it must begin bank-aligned
    within its unit; close the unit when it wouldn't (e.g. a full slot
    following the half-used 256 bank in partials-first order)."""
    units = []
    cur, banks, cw = [], 0, 0
    for s in slots:
        nb = 1 if s[4] else 0  # the start=False 128 slot shares its bank
        if cur and ((banks + nb) > UNIT // 512 or (s[4] and cw % 512 != 0)):
            units.append(cur)
            cur, banks, cw = [], 0, 0
        cur.append(s)
        banks += nb
        cw += s[2]
    if cur:
        units.append(cur)
    return units


def _build_program():
    import concourse.bacc as bacc
    import concourse.tile as tile
    from concourse import mybir

    fp32 = mybir.dt.float32
    bf16 = mybir.dt.bfloat16
    EXP = mybir.ActivationFunctionType.Exp

    PTW = 512 * NTS  # 16384: per-tile P^T buffer, slot a at column 512*a

    nc = bacc.Bacc("TRN2", target_bir_lowering=False, debug=False)
    xt_ap = nc.dram_tensor("xt", [128, NTT, KCH, 512], bf16, kind="ExternalInput").ap()
    w_ap = nc.dram_tensor("w", [128, KCH, 192], bf16, kind="ExternalInput").ap()
    mask_ap = nc.dram_tensor("mask", [128, 128], bf16, kind="ExternalInput").ap()
    o_ap = nc.dram_tensor("o", [128, NTS, HEAD], fp32, kind="ExternalOutput").ap()

    with tile.TileContext(nc) as tc:
        with (
            tc.tile_pool(name="consts", bufs=1) as consts,
            tc.tile_pool(name="xp", bufs=4) as xp,
            tc.tile_pool(name="ptp", bufs=4) as ptp,
            tc.tile_pool(name="outs", bufs=4) as outs,
            tc.tile_pool(name="ps_s", bufs=2, space="PSUM") as ps_s,
            tc.tile_pool(name="ps_kq", bufs=1, space="PSUM") as ps_kq,
            tc.tile_pool(name="ps_vpv", bufs=1, space="PSUM") as ps_vpv,
        ):
            # ---------- constants / warm-up ----------
            # scratch for PE warm-up: memset'd early on gpsimd, matmul'd in
            # a burst so the HAM clock gate ramps before real work. The
            # dummies alternate PSUM halves so consecutive matmuls don't
            # serialize on a same-element WAW hazard.
            ws = consts.tile([128, 128], bf16, tag="wmm")
            nc.gpsimd.memset(ws, 0.0)

            w_sb = consts.tile([128, KCH, 192], bf16, tag="w")
            mask_sb = consts.tile([128, 128], bf16, tag="mask")
            with tc.high_priority():
                # first two k-chunks of the weights land first (sync queue,
                # ahead of the x stream) so KQ(0) can start as soon as the
                # first x chunk is in
                nc.sync.dma_start(out=w_sb[:, 0:2, :], in_=w_ap[:, 0:2, :])
                nc.scalar.dma_start(out=w_sb[:, 2:8, :], in_=w_ap[:, 2:8, :])
                nc.scalar.dma_start(out=mask_sb, in_=mask_ap)
            # V with ones column: slot aa is [128, 65], col 64 preset to 1.0
            vt_sb = consts.tile([128, NTS, 65], bf16, tag="vt")
            nc.gpsimd.memset(vt_sb[:, :, 64:65], 1.0)
            kq_sb = consts.tile([128, T], bf16, tag="kq")
            qk_sb = consts.tile([128, T], bf16, tag="qk")
            # warm the exp table so ACT_TABLE_LOAD doesn't hit the first unit
            warm = consts.tile([128, 1], bf16, tag="warm")
            nc.scalar.activation(warm, ws[:, 0:1], EXP, scale=1.0)

            wps = ps_kq.tile([128, 256], fp32, tag="kq", name="warmps")
            for i in range(WARM_MMS):
                h = (i % 2) * 128
                nc.tensor.matmul(wps[:, h:h + 128], ws, ws, start=True,
                                 stop=True, skip_group_check=True)

            xt_t = {}

            def dma_x(j):
                xt_t[j] = xp.tile([128, KCH, 512], bf16, tag="x", name=f"xt{j}")
                if j == 0:
                    # geometric split so KQ(0) starts on the first chunk
                    # asap; spread across queues so the descriptor writes
                    # don't serialize on one sequencer (gpsimd's queue is
                    # free earliest).
                    for q, (lo, hi) in zip(
                        (nc.gpsimd, nc.sync, nc.sync, nc.sync),
                        ((0, 1), (1, 2), (2, 4), (4, 8)),
                    ):
                        q.dma_start(
                            out=xt_t[j][:, lo:hi, :],
                            in_=xt_ap[:, j, lo:hi, :],
                        )
                else:
                    nc.sync.dma_start(out=xt_t[j], in_=xt_ap[:, j, :, :])

            kq_ps = {}
            v_ps = {}

            def emit_kq_half(j, half):
                """Half of QK^T tile j (4 k-chunks); cast + shifts on half 1."""
                if half == 0:
                    kq_ps[j] = ps_kq.tile([128, 512], fp32, tag="kq", name=f"kq{j}")
                for k in range(4 * half, 4 * half + 4):
                    nc.tensor.matmul(
                        kq_ps[j],
                        w_sb[:, k, 0:128],
                        xt_t[j][:, k, :],
                        start=(k == 0),
                        stop=(k == KCH - 1),
                    )
                if half == 1:
                    jsl = slice(j * 512, (j + 1) * 512)
                    nc.vector.tensor_copy(kq_sb[:, jsl], kq_ps[j])
                    if j >= 2:
                        # K^T to base 0 (row-tile T0 pair), Q^T to base 64
                        # (T8); separate queues so the descriptor writes
                        # overlap. Tiles 0/1 build qk_sb via emit_kq2(): a
                        # second projection pass with swapped layout, so the
                        # first score units don't wait on sbuf-sbuf DMA
                        # latency.
                        nc.sync.dma_start(
                            out=qk_sb[0:64, jsl], in_=kq_sb[64:128, jsl]
                        )
                        nc.gpsimd.dma_start(
                            out=qk_sb[64:128, jsl], in_=kq_sb[0:64, jsl]
                        )

            qk_ps = {}

            def emit_kq2_half(j, half):
                """Swapped-layout projection [K^T@0; Q^T@64] for tile j<2.

                Two column-tiled half matmuls per chunk (Wk -> out[0:64],
                Wq -> out[64:128]) run concurrently on distinct PE column
                groups; the cast writes qk_sb directly, replacing the
                shift-DMA copies on the startup critical path."""
                if half == 0:
                    qk_ps[j] = ps_s.tile([128, 512], fp32, tag="s", name=f"qk2_{j}")
                for k in range(4 * half, 4 * half + 4):
                    nc.tensor.matmul(
                        qk_ps[j][0:64, :],
                        w_sb[:, k, 64:128],
                        xt_t[j][:, k, :],
                        start=(k == 0),
                        stop=(k == KCH - 1),
                        skip_group_check=True,
                    )
                    nc.tensor.matmul(
                        qk_ps[j][64:128, :],
                        w_sb[:, k, 0:64],
                        xt_t[j][:, k, :],
                        start=(k == 0),
                        stop=(k == KCH - 1),
                        skip_group_check=True,
                    )
                if half == 1:
                    jsl = slice(j * 512, (j + 1) * 512)
                    nc.vector.tensor_copy(qk_sb[:, jsl], qk_ps[j])

            def emit_v_half(j, half):
                """Half of V tile j (4 k-chunks); copy-with-ones on half 1."""
                if half == 0:
                    v_ps[j] = ps_vpv.tile([128, 4, 64], fp32, tag="vpv", name=f"v{j}")
                for k in range(4 * half, 4 * half + 4):
                    for q in range(4):
                        nc.tensor.matmul(
                            v_ps[j][:, q, :],
                            xt_t[j][:, k, q * 128:(q + 1) * 128],
                            w_sb[:, k, 128:192],
                            start=(k == 0 and q == 0),
                            stop=(k == KCH - 1),
                            skip_group_check=True,
                        )
                if half == 1:
                    nc.vector.tensor_copy(vt_sb[:, 4 * j:4 * j + 4, 0:64], v_ps[j])

            pt = {}
            par = [0]  # global row-tile parity, alternates across slots

            def emit_unit(j, unit):
                uw = sum(s[2] for s in unit)
                base = unit[0][3]
                psu = ps_s.tile([128, UNIT], fp32, tag="s")
                with tc.high_priority():
                    for (a, t0, w, off, start) in unit:
                        if start:
                            p = par[0]
                            par[0] ^= 1
                        # a start=False slot shares its opener's PSUM bank:
                        # keep it on the same row tile so the two matmuls
                        # never touch that bank concurrently.
                        if p == 0:
                            lhsT = qk_sb[0:64, a * 128:(a + 1) * 128]
                            rhs = kq_sb[0:64, t0:t0 + w]
                        else:
                            lhsT = kq_sb[64:128, a * 128:(a + 1) * 128]
                            rhs = qk_sb[64:128, t0:t0 + w]
                        nc.tensor.matmul(
                            psu[:, off - base:off - base + w],
                            lhsT,
                            rhs,
                            start=start,
                            stop=True,
                            skip_group_check=True,
                        )
                nc.scalar.activation(
                    pt[j][:, base:base + uw], psu[:, 0:uw], EXP, scale=0.125
                )

            def emit_masks(j, slots):
                # diagonal 128-block is the first 128 columns of each of the
                # four slots for chunks 4j..4j+3
                for (a, t0, w, off, start) in slots:
                    if a < 4 * j:
                        continue
                    nc.vector.tensor_mul(
                        pt[j][:, off:off + 128], pt[j][:, off:off + 128], mask_sb
                    )

            ogs = {}
            po_t = {}

            def chain_seg(i, pool, lo, hi):
                """Matmuls lo..hi-1 of PV chain i (chain length i+1)."""
                jj = i // 4
                smap = {s[0]: s for s in _tile_slots(jj, partials_first=(jj == NTT - 1))}
                if lo == 0:
                    tag = {id(ps_vpv): "vpv", id(ps_kq): "kq", id(ps_s): "s"}[id(pool)]
                    po_t[i] = pool.tile([128, 65], fp32, tag=tag, name=f"po{i}")
                po = po_t[i]
                for aa in range(lo, hi):
                    (_, t0, _, off, _) = smap[aa]
                    col = off + 128 * i - t0
                    nc.tensor.matmul(
                        po,
                        pt[jj][:, col:col + 128],
                        vt_sb[:, aa, :],
                        start=(aa == 0),
                        stop=(aa == i),
                    )

            def chain_fin(i):
                jj = i // 4
                po_s = outs.tile([128, 65], fp32, tag="po_s", name=f"pos{i}")
                nc.vector.tensor_copy(po_s, po_t[i])
                dr = outs.tile([128, 1], fp32, tag="dr")
                nc.vector.reciprocal(dr, po_s[:, 64:65])
                nc.vector.tensor_scalar_mul(ogs[jj][:, i % 4, :], po_s[:, 0:64], dr)

            def chain_work(i, pool):
                """Chain i as a list of small work items."""
                items = []
                for lo in range(0, i + 1, SEG):
                    hi = min(lo + SEG, i + 1)
                    items.append(lambda i=i, pool=pool, lo=lo, hi=hi:
                                 chain_seg(i, pool, lo, hi))
                items.append(lambda i=i: chain_fin(i))
                return items

            # ---------- pipeline ----------
            # startup: x tiles 0-3 + projections for tiles 0 and 1 while the
            # exp stream hasn't started (PE would otherwise idle on DMA).
            with tc.high_priority():
                for j in range(4):
                    dma_x(j)
            for h in range(2):
                emit_kq_half(0, h)
                emit_kq2_half(0, h)
            for h in range(2):
                emit_kq_half(1, h)
                emit_kq2_half(1, h)
            for h in range(2):
                emit_v_half(0, h)

            for j in range(NTT):
                slots = _tile_slots(j, partials_first=(j == NTT - 1))
                units = _units(slots)
                pt[j] = ptp.tile([128, PTW], bf16, tag="pt", name=f"pt{j}")

                # PE side-work interleaved between score units so the tensor
                # engine never idles (keeps the p-state ramped) and the scalar
                # engine is never blocked behind a stalled PE queue. KQ/V for
                # tile j+2 run during tile j (they were prefetched two tiles
                # ahead at startup), chains of tile j-1 fill the rest.
                work = []
                if j == 0:
                    for h in range(2):
                        work.append(lambda h=h: emit_v_half(1, h))
                if j + 2 < NTT:
                    for h in range(2):
                        work.append(lambda j=j, h=h: emit_kq_half(j + 2, h))
                    for h in range(2):
                        work.append(lambda j=j, h=h: emit_v_half(j + 2, h))
                if j + 4 < NTT:
                    work.append(lambda j=j: dma_x(j + 4))
                if j >= 1:
                    ogs[j - 1] = outs.tile(
                        [128, 4, 64], fp32, tag="og", name=f"og{j - 1}"
                    )
                    for i in range(4 * (j - 1), 4 * j):
                        work.extend(chain_work(i, ps_vpv))
                    work.append(lambda j=j: nc.sync.dma_start(
                        out=o_ap[:, 4 * (j - 1):4 * j, :], in_=ogs[j - 1]
                    ))

                # units in pairs: consecutive score-matmul runs share one
                # 64x128-mode switch and overlap T0/T8 across the boundary
                done = 0
                for n in range(0, len(units), 2):
                    pair = units[n:n + 2]
                    for u in pair:
                        emit_unit(j, u)
                    hi = (n + len(pair)) * len(work) // len(units)
                    while done < hi:
                        work[done]()
                        done += 1
                emit_masks(j, slots)

            # tail: chains of tile 7, two banks in parallel (vpv + kq pools)
            ogs[NTT - 1] = outs.tile([128, 4, 64], fp32, tag="og", name="og7")
            for n, i in enumerate(range(4 * (NTT - 1), 4 * NTT)):
                pool = (ps_vpv, ps_kq, ps_vpv, ps_kq)[n]
                for item in chain_work(i, pool):
                    item()
            nc.sync.dma_start(out=o_ap[:, 4 * (NTT - 1):4 * NTT, :], in_=ogs[NTT - 1])

    nc.compile()
    return nc


def _get_nc():
    if "nc" not in _CACHE:
        _CACHE["nc"] = _build_program()
    return _CACHE["nc"]


def _prep_inputs(x, W):
    """Host-side packing shared by kernel() and test harnesses."""
    x = np.asarray(x, dtype=np.float32)
    W = np.asarray(W, dtype=np.float32)
    assert x.shape == (B, T, EMB) and W.shape == (EMB, 3 * HEAD)
    # [B, 128, KCH, T]: partition p of chunk k holds x[b, :, 128k+p]
    xt = np.ascontiguousarray(
        x.transpose(0, 2, 1)
        .reshape(B, KCH, 128, NTT, 512)
        .transpose(0, 2, 3, 1, 4)
    ).astype(BF16)
    w_perm = np.concatenate(
        [W[:, 64:128], W[:, 0:64], W[:, 128:192]], axis=1
    )
    w_r = np.ascontiguousarray(
        w_perm.reshape(KCH, 128, 3 * HEAD)
    ).transpose(1, 0, 2).astype(BF16)
    w_r = np.ascontiguousarray(w_r)
    mask = np.triu(np.ones((128, 128), np.float32)).astype(BF16)
    return xt, w_r, mask


def kernel(x, W):
    from concourse.bass_utils import run_bass_kernel_spmd

    xt, w_r, mask = _prep_inputs(x, W)
    nc = _get_nc()
    in_maps = [{"xt": xt[b], "w": w_r, "mask": mask} for b in range(B)]
    res = run_bass_kernel_spmd(nc, in_maps, list(range(B)))
    # o[p, i, c] = out[128*i + p, c]
    return np.stack(
        [
            res.results[b]["o"].transpose(1, 0, 2).reshape(T, HEAD)
            for b in range(B)
        ]
    ).astype(np.float32)


# revision 15
# speedup vs baseline: 1.0131x; 1.0131x over previous
"""Single-head causal attention (B=8, T=4096, EMB=1024, HEAD=64) on 8 trn2 cores.

Strategy: data-parallel over batch, one batch element per NeuronCore.

The per-core kernel is scalar-exp-bound (softmax exp runs only on the Scalar
engine at 1 col/cycle: causal T^2/2 elements = 67584 cols ~ 56us @1.2GHz), so
the whole kernel is organized to start exp as early as possible and keep the
Scalar engine 100% fed:

  - x is host-packed as [128, KCH=8, T] and DMA'd per t-tile j (512 cols,
    1MB, ~2.9us each) across multiple issue queues.
  - Per tile j: QK^T[128, 512] (8 k-chunk matmuls into 1 PSUM bank; rows
    0:63 = Q^T, rows 64:127 = K^T), cast to bf16, then TWO shift DMAs build
    qk_sb: rows 0:63 = K^T copy, rows 64:127 = Q^T copy. This gives a
    (K^T, Q^T) operand pair at BOTH partition bases 0 and 64, so score
    matmuls alternate between PE row-tiles T0 (base 0) and T8 (base 64)
    and run concurrently (64x128 row tiling, ~2x effective rate).
  - V[4 subtiles, 64] (32 matmuls, stationary = xt chunk) + copy-with-ones.
  - Scores for s-chunks a<=4j+3 vs t-tile j stream into [128,<=1536] PSUM
    units (3 banks x2 buffered); ScalarE exp's each unit (scale=1/8 folded),
    bf16 out into a per-tile P^T buffer (exact causal widths). Diagonal
    128x128 blocks masked by 0/1 multiply after exp.
  - PV: per t-subtile i, chain of i+1 matmuls (stationary P^T slice
    [128,128], moving V-with-ones [128,65]) accumulates [O|rowsum]; chains
    are emitted in small segments interleaved between score units so score
    matmuls never queue behind a long chain. out = O * reciprocal(rowsum).
  - A burst of dummy matmuls at program start ramps the PE HAM clock gate
    to 2.4GHz before the first real matmul.

PSUM budget (8 banks): scores 2x[128,1536] (6) + KQ [128,512] (1) + V/PV
shared (1).
"""

import numpy as np
import ml_dtypes

B, T, EMB, HEAD = 8, 4096, 1024, 64
KCH = EMB // 128          # 8 contraction chunks
NTT = T // 512            # 8 t-tiles of 512
NTS = T // 128            # 32 t-subtiles / s-chunks of 128
UNIT = 1536               # max score-unit width (3 PSUM banks)
WARM_MMS = 12             # PE clock-gate warm-up matmuls
SEG = 6                   # PV chain segment length (matmuls)
BF16 = ml_dtypes.bfloat16

_CACHE = {}


def _tile_slots(j, partials_first=False):
    """[(a, t0, w, pt_off, start_flag), ...] score slots for t-tile j.

    Exact causal widths, packed so every slot either begins at a PSUM bank
    boundary (start=True allowed: clearing the bank wipes nothing live) or
    is the 128-wide slot sharing the bank opened by the 384 one
    (start=False: its elements' has_written bits were cleared by that
    start). Widths 384,128 share a bank; 256 gets its own (tail unused);
    fulls are 512 each. For the last tile the partials go FIRST so the
    final PV chains are gated by the last full unit, not the partials
    (safe only there: partials need this tile's own K-copy)."""
    parts = [(4 * j + 1, True), (4 * j + 3, False), (4 * j + 2, True)]
    fulls = [(a, True) for a in range(4 * j + 1)]
    order = parts + fulls if partials_first else fulls + parts
    slots = []
    off = 0
    for a, flag in order:
        t0 = max(512 * j, 128 * a)
        w = 512 * (j + 1) - t0
        slots.append((a, t0, w, off, flag))
        off += w
    return slots


def _units(slots):
    """Greedy-pack slots into units of <= UNIT psum columns, whole banks.

    A start=True slot opens a PSUM bank, so it must begin bank-aligned
    within its unit; close the unit when it wouldn't (e.g. a full slot
    following the half-used 256 bank in partials-first order)."""
    units = []
    cur, banks, cw = [], 0, 0
    for s in slots:
        nb = 1 if s[4] else 0  # the start=False 128 slot shares its bank
        if cur and ((banks + nb) > UNIT // 512 or (s[4] and cw % 512 != 0)):
            units.append(cur)
            cur, banks, cw = [], 0, 0
        cur.append(s)
        banks += nb
        cw += s[2]
    if cur:
        units.append(cur)
    return units


def _build_program():
    import concourse.bacc as bacc
    import concourse.tile as tile
    from concourse import mybir

    fp32 = mybir.dt.float32
    bf16 = mybir.dt.bfloat16
    EXP = mybir.ActivationFunctionType.Exp

    PTW = 512 * NTS  # 16384: per-tile P^T buffer, slot a at column 512*a

    nc = bacc.Bacc("TRN2", target_bir_lowering=False, debug=False)
    xt_ap = nc.dram_tensor("xt", [128, NTT, KCH, 512], bf16, kind="ExternalInput").ap()
    w_ap = nc.dram_tensor("w", [128, KCH, 192], bf16, kind="ExternalInput").ap()
    mask_ap = nc.dram_tensor("mask", [128, 128], bf16, kind="ExternalInput").ap()
    o_ap = nc.dram_tensor("o", [128, NTS, HEAD], fp32, kind="ExternalOutput").ap()

    with tile.TileContext(nc) as tc:
        with (
            tc.tile_pool(name="consts", bufs=1) as consts,
            tc.tile_pool(name="xp", bufs=4) as xp,
            tc.tile_pool(name="ptp", bufs=4) as ptp,
            tc.tile_pool(name="outs", bufs=4) as outs,
            tc.tile_pool(name="ps_s", bufs=2, space="PSUM") as ps_s,
            tc.tile_pool(name="ps_kq", bufs=1, space="PSUM") as ps_kq,
            tc.tile_pool(name="ps_vpv", bufs=1, space="PSUM") as ps_vpv,
        ):
            # ---------- constants / warm-up ----------
            # scratch for PE warm-up: memset'd early on gpsimd, matmul'd in
            # a burst so the HAM clock gate ramps before real work. The
            # dummies alternate PSUM halves so consecutive matmuls don't
            # serialize on a same-element WAW hazard.
            ws = consts.tile([128, 128], bf16, tag="wmm")
            nc.gpsimd.memset(ws, 0.0)

            w_sb = consts.tile([128, KCH, 192], bf16, tag="w")
            mask_sb = consts.tile([128, 128], bf16, tag="mask")
            with tc.high_priority():
                # first two k-chunks of the weights land first (sync queue,
                # ahead of the x stream) so KQ(0) can start as soon as the
                # first x chunk is in
                nc.sync.dma_start(out=w_sb[:, 0:2, :], in_=w_ap[:, 0:2, :])
                nc.scalar.dma_start(out=w_sb[:, 2:8, :], in_=w_ap[:, 2:8, :])
                nc.scalar.dma_start(out=mask_sb, in_=mask_ap)
            # V with ones column: slot aa is [128, 65], col 64 preset to 1.0
            vt_sb = consts.tile([128, NTS, 65], bf16, tag="vt")
            nc.gpsimd.memset(vt_sb[:, :, 64:65], 1.0)
            kq_sb = consts.tile([128, T], bf16, tag="kq")
            qk_sb = consts.tile([128, T], bf16, tag="qk")
            # warm the exp table so ACT_TABLE_LOAD doesn't hit the first unit
            warm = consts.tile([128, 1], bf16, tag="warm")
            nc.scalar.activation(warm, ws[:, 0:1], EXP, scale=1.0)

            wps = ps_kq.tile([128, 256], fp32, tag="kq", name="warmps")
            for i in range(WARM_MMS):
                h = (i % 2) * 128
                nc.tensor.matmul(wps[:, h:h + 128], ws, ws, start=True,
                                 stop=True, skip_group_check=True)

            xt_t = {}

            def dma_x(j):
                xt_t[j] = xp.tile([128, KCH, 512], bf16, tag="x", name=f"xt{j}")
                if j == 0:
                    # geometric split so KQ(0) starts on the first chunk
                    # asap; spread across queues so the descriptor writes
                    # don't serialize on one sequencer (gpsimd's queue is
                    # free earliest).
                    for q, (lo, hi) in zip(
                        (nc.gpsimd, nc.sync, nc.sync, nc.sync),
                        ((0, 1), (1, 2), (2, 4), (4, 8)),
                    ):
                        q.dma_start(
                            out=xt_t[j][:, lo:hi, :],
                            in_=xt_ap[:, j, lo:hi, :],
                        )
                else:
                    nc.sync.dma_start(out=xt_t[j], in_=xt_ap[:, j, :, :])

            kq_ps = {}
            v_ps = {}

            def emit_kq_half(j, half):
                """Half of QK^T tile j (4 k-chunks); cast + shifts on half 1."""
                if half == 0:
                    kq_ps[j] = ps_kq.tile([128, 512], fp32, tag="kq", name=f"kq{j}")
                for k in range(4 * half, 4 * half + 4):
                    nc.tensor.matmul(
                        kq_ps[j],
                        w_sb[:, k, 0:128],
                        xt_t[j][:, k, :],
                        start=(k == 0),
                        stop=(k == KCH - 1),
                    )
                if half == 1:
                    jsl = slice(j * 512, (j + 1) * 512)
                    nc.vector.tensor_copy(kq_sb[:, jsl], kq_ps[j])

            qk_ps = {}

            def emit_kq2_half(j, half):
                """Swapped-layout projection [K^T@0; Q^T@64] for tile j.

                Two column-tiled half matmuls per chunk (Wk -> out[0:64],
                Wq -> out[64:128]) run concurrently on distinct PE column
                groups; the cast writes qk_sb directly. Purely compute-side
                (no sbuf-sbuf shift DMA), so score-unit readiness is
                deterministic for the Tile scheduler."""
                if half == 0:
                    qk_ps[j] = ps_kq.tile([128, 512], fp32, tag="kq", name=f"qk2_{j}")
                for k in range(4 * half, 4 * half + 4):
                    nc.tensor.matmul(
                        qk_ps[j][0:64, :],
                        w_sb[:, k, 64:128],
                        xt_t[j][:, k, :],
                        start=(k == 0),
                        stop=(k == KCH - 1),
                        skip_group_check=True,
                    )
                    nc.tensor.matmul(
                        qk_ps[j][64:128, :],
                        w_sb[:, k, 0:64],
                        xt_t[j][:, k, :],
                        start=(k == 0),
                        stop=(k == KCH - 1),
                        skip_group_check=True,
                    )
                if half == 1:
                    jsl = slice(j * 512, (j + 1) * 512)
                    nc.vector.tensor_copy(qk_sb[:, jsl], qk_ps[j])

            def emit_v_half(j, half):
                """Half of V tile j (4 k-chunks); copy-with-ones on half 1."""
                if half == 0:
                    v_ps[j] = ps_vpv.tile([128, 4, 64], fp32, tag="vpv", name=f"v{j}")
                for k in range(4 * half, 4 * half + 4):
                    for q in range(4):
                        nc.tensor.matmul(
                            v_ps[j][:, q, :],
                            xt_t[j][:, k, q * 128:(q + 1) * 128],
                            w_sb[:, k, 128:192],
                            start=(k == 0 and q == 0),
                            stop=(k == KCH - 1),
                            skip_group_check=True,
                        )
                if half == 1:
                    nc.vector.tensor_copy(vt_sb[:, 4 * j:4 * j + 4, 0:64], v_ps[j])

            pt = {}
            par = [0]  # global row-tile parity, alternates across slots

            def emit_unit(j, unit):
                uw = sum(s[2] for s in unit)
                base = unit[0][3]
                psu = ps_s.tile([128, UNIT], fp32, tag="s")
                with tc.high_priority():
                    for (a, t0, w, off, start) in unit:
                        if start:
                            p = par[0]
                            par[0] ^= 1
                        # a start=False slot shares its opener's PSUM bank:
                        # keep it on the same row tile so the two matmuls
                        # never touch that bank concurrently.
                        if p == 0:
                            lhsT = qk_sb[0:64, a * 128:(a + 1) * 128]
                            rhs = kq_sb[0:64, t0:t0 + w]
                        else:
                            lhsT = kq_sb[64:128, a * 128:(a + 1) * 128]
                            rhs = qk_sb[64:128, t0:t0 + w]
                        nc.tensor.matmul(
                            psu[:, off - base:off - base + w],
                            lhsT,
                            rhs,
                            start=start,
                            stop=True,
                            skip_group_check=True,
                        )
                nc.scalar.activation(
                    pt[j][:, base:base + uw], psu[:, 0:uw], EXP, scale=0.125
                )

            def emit_masks(j, slots):
                # diagonal 128-block is the first 128 columns of each of the
                # four slots for chunks 4j..4j+3
                for (a, t0, w, off, start) in slots:
                    if a < 4 * j:
                        continue
                    nc.vector.tensor_mul(
                        pt[j][:, off:off + 128], pt[j][:, off:off + 128], mask_sb
                    )

            ogs = {}
            po_t = {}

            def chain_seg(i, pool, lo, hi):
                """Matmuls lo..hi-1 of PV chain i (chain length i+1)."""
                jj = i // 4
                smap = {s[0]: s for s in _tile_slots(jj, partials_first=(jj == NTT - 1))}
                if lo == 0:
                    tag = {id(ps_vpv): "vpv", id(ps_kq): "kq", id(ps_s): "s"}[id(pool)]
                    po_t[i] = pool.tile([128, 65], fp32, tag=tag, name=f"po{i}")
                po = po_t[i]
                for aa in range(lo, hi):
                    (_, t0, _, off, _) = smap[aa]
                    col = off + 128 * i - t0
                    nc.tensor.matmul(
                        po,
                        pt[jj][:, col:col + 128],
                        vt_sb[:, aa, :],
                        start=(aa == 0),
                        stop=(aa == i),
                    )

            def chain_fin(i):
                jj = i // 4
                po_s = outs.tile([128, 65], fp32, tag="po_s", name=f"pos{i}")
                nc.vector.tensor_copy(po_s, po_t[i])
                dr = outs.tile([128, 1], fp32, tag="dr")
                nc.vector.reciprocal(dr, po_s[:, 64:65])
                nc.vector.tensor_scalar_mul(ogs[jj][:, i % 4, :], po_s[:, 0:64], dr)

            def chain_work(i, pool):
                """Chain i as a list of small work items."""
                items = []
                for lo in range(0, i + 1, SEG):
                    hi = min(lo + SEG, i + 1)
                    items.append(lambda i=i, pool=pool, lo=lo, hi=hi:
                                 chain_seg(i, pool, lo, hi))
                items.append(lambda i=i: chain_fin(i))
                return items

            # ---------- pipeline ----------
            # startup: x tiles 0-3 + projections for tiles 0 and 1 while the
            # exp stream hasn't started (PE would otherwise idle on DMA).
            with tc.high_priority():
                for j in range(4):
                    dma_x(j)
            for h in range(2):
                emit_kq_half(0, h)
                emit_kq2_half(0, h)
            for h in range(2):
                emit_kq_half(1, h)
                emit_kq2_half(1, h)
            for h in range(2):
                emit_v_half(0, h)

            for j in range(NTT):
                slots = _tile_slots(j, partials_first=(j == NTT - 1))
                units = _units(slots)
                pt[j] = ptp.tile([128, PTW], bf16, tag="pt", name=f"pt{j}")

                # PE side-work interleaved between score units so the tensor
                # engine never idles (keeps the p-state ramped) and the scalar
                # engine is never blocked behind a stalled PE queue. KQ/V for
                # tile j+2 run during tile j (they were prefetched two tiles
                # ahead at startup), chains of tile j-1 fill the rest.
                work = []
                if j == 0:
                    for h in range(2):
                        work.append(lambda h=h: emit_v_half(1, h))
                if j + 2 < NTT:
                    for h in range(2):
                        work.append(lambda j=j, h=h: emit_kq_half(j + 2, h))
                    for h in range(2):
                        work.append(lambda j=j, h=h: emit_kq2_half(j + 2, h))
                    for h in range(2):
                        work.append(lambda j=j, h=h: emit_v_half(j + 2, h))
                if j + 4 < NTT:
                    work.append(lambda j=j: dma_x(j + 4))
                if j >= 1:
                    ogs[j - 1] = outs.tile(
                        [128, 4, 64], fp32, tag="og", name=f"og{j - 1}"
                    )
                    for i in range(4 * (j - 1), 4 * j):
                        work.extend(chain_work(i, ps_vpv))
                    work.append(lambda j=j: nc.sync.dma_start(
                        out=o_ap[:, 4 * (j - 1):4 * j, :], in_=ogs[j - 1]
                    ))

                # units in pairs: consecutive score-matmul runs share one
                # 64x128-mode switch and overlap T0/T8 across the boundary
                done = 0
                for n in range(0, len(units), 2):
                    pair = units[n:n + 2]
                    for u in pair:
                        emit_unit(j, u)
                    hi = (n + len(pair)) * len(work) // len(units)
                    while done < hi:
                        work[done]()
                        done += 1
                emit_masks(j, slots)

            # tail: chains of tile 7, two banks in parallel (vpv + kq pools)
            ogs[NTT - 1] = outs.tile([128, 4, 64], fp32, tag="og", name="og7")
            for n, i in enumerate(range(4 * (NTT - 1), 4 * NTT)):
                pool = (ps_vpv, ps_kq, ps_vpv, ps_kq)[n]
                for item in chain_work(i, pool):
                    item()
            nc.sync.dma_start(out=o_ap[:, 4 * (NTT - 1):4 * NTT, :], in_=ogs[NTT - 1])

    nc.compile()
    return nc


def _get_nc():
    if "nc" not in _CACHE:
        _CACHE["nc"] = _build_program()
    return _CACHE["nc"]


def _prep_inputs(x, W):
    """Host-side packing shared by kernel() and test harnesses."""
    x = np.asarray(x, dtype=np.float32)
    W = np.asarray(W, dtype=np.float32)
    assert x.shape == (B, T, EMB) and W.shape == (EMB, 3 * HEAD)
    # [B, 128, KCH, T]: partition p of chunk k holds x[b, :, 128k+p]
    xt = np.ascontiguousarray(
        x.transpose(0, 2, 1)
        .reshape(B, KCH, 128, NTT, 512)
        .transpose(0, 2, 3, 1, 4)
    ).astype(BF16)
    w_perm = np.concatenate(
        [W[:, 64:128], W[:, 0:64], W[:, 128:192]], axis=1
    )
    w_r = np.ascontiguousarray(
        w_perm.reshape(KCH, 128, 3 * HEAD)
    ).transpose(1, 0, 2).astype(BF16)
    w_r = np.ascontiguousarray(w_r)
    mask = np.triu(np.ones((128, 128), np.float32)).astype(BF16)
    return xt, w_r, mask


def kernel(x, W):
    from concourse.bass_utils import run_bass_kernel_spmd

    xt, w_r, mask = _prep_inputs(x, W)
    nc = _get_nc()
    in_maps = [{"xt": xt[b], "w": w_r, "mask": mask} for b in range(B)]
    res = run_bass_kernel_spmd(nc, in_maps, list(range(B)))
    # o[p, i, c] = out[128*i + p, c]
    return np.stack(
        [
            res.results[b]["o"].transpose(1, 0, 2).reshape(T, HEAD)
            for b in range(B)
        ]
    ).astype(np.float32)
